# revision 22
# baseline (speedup 1.0000x reference)
"""ChannelPatchEmbed kernel for Trainium2 (8 NeuronCores, batch-parallel).

Computation: concat 8 single-feature channels -> each 512x512 image goes
through the SAME 1->96 conv (4x4 patches, stride 4) + bias.
Output: [8, 768, 128, 128] f32.

Strategy per core (1 batch sample per core):
  - GEMM formulation: K = (j, i, c) = 4*4*8 = 128 on the contraction
    partitions, block-diagonal stationary S (6 chunks of 16 output
    channels x 8 input channels = 128 M) -> one K=128 matmul yields 128
    output channels per 512-position window.
  - Patch-decimated input layout (host pre-shuffle): x'[32j+8i+c,
    hp*128+m] = image[c, 4*hp+i, 4*m+j].  Every pixel appears exactly
    once, so the image loads from HBM ONCE (4.2 MB bf16) as full
    128-partition DMAs with contiguous multi-KB descriptors, and the
    matmul rhs is a contiguous 512-wide SBUF slice (no stride-4 read,
    no shifted on-chip copies).
  - Each store is issued as two column halves on the ACT and POOL
    HWDGE rings concurrently: per-tile completion latency halves, so
    the 5 staging buffers recycle ~2x faster — relieving slow-mode
    jams where store completions gate evictions.
  - bf16: inputs and W are host-cast to bf16 (tolerance 2e-2, measured
    rel err ~3.3e-3); PE runs at full rate, PSUM accumulates f32.
  - Bias is fused into wide 4-bank PSUM->SBUF evictions (ACT/DVE
    alternating); stores are full-width 128-partition DMAs alternating
    between the ACT and POOL HWDGE rings with 8-16 KB descriptors.
  - Graduated stage sizes (STAGES windows of 512 positions) shorten the
    load ramp at the start; stage bufs=5 caps the evicted-but-unstored
    backlog so the post-compute store drain stays short while still
    absorbing HBM-arbitration jitter from the other 7 cores.  One R
    buffer per stage (rin bufs=6) lets every load prefetch arbitrarily
    far ahead, cushioning runs where the load phase loses HBM
    arbitration to the other cores.
"""

import sys

import numpy as np

if "/opt/trn_rl_repo" not in sys.path:
    sys.path.insert(0, "/opt/trn_rl_repo")

import ml_dtypes

import concourse.bacc as bacc
import concourse.mybir as mybir
import concourse.tile as tile
from concourse.bass_utils import run_bass_kernel_spmd

F32 = mybir.dt.float32
BF16 = mybir.dt.bfloat16

N_CORES = 8
C = 8            # input channels per sample (3 rgb + 4 hs + 1 dem)
H = 512          # image height/width
PATCH = 4
HP = H // PATCH  # 128 patches per side
NPOS = HP * HP   # 16384 output positions per image
EMBED = 96
CHUNKS = 6       # 96*8 = 768 output channels in chunks of 128
# windows (512 output positions each) per pipeline stage; 32 windows
# total.  Small first stage -> first store flows after ~1us of load;
# tapered last stages -> short drain after the final eviction.
STAGES = [2, 4, 8, 8, 6, 4]

_NC_CACHE = None


def _build_nc():
    nc = bacc.Bacc("TRN2", target_bir_lowering=False, detect_race_conditions=False)
    x = nc.dram_tensor("x", [128, NPOS], BF16, kind="ExternalInput")
    s = nc.dram_tensor("s", [128, CHUNKS * 128], BF16, kind="ExternalInput")
    bias = nc.dram_tensor("bias", [128, 128], F32, kind="ExternalInput")
    y = nc.dram_tensor("y", [C * EMBED, HP, HP], F32, kind="ExternalOutput")

    y_v = y.rearrange("ch h w -> ch (h w)")  # [768, 16384]

    with tile.TileContext(nc) as tc:
        with (
            tc.tile_pool(name="const", bufs=1) as const_pool,
            tc.tile_pool(name="rin", bufs=6) as r_pool,
            tc.tile_pool(name="stage", bufs=5) as stage_pool,
            tc.tile_pool(name="psum", bufs=2, space="PSUM") as psum_pool,
        ):
            # Pad so every subsequent tile is 512 B-aligned: the framework's
            # const-scalar region ends at +128 B, and SDMA's sub-512B write
            # path does RMW on 512 B granules — cross-tensor granule sharing
            # between concurrent DMA writers would corrupt data.
            _align_pad = const_pool.tile([128, 96], F32, tag="align_pad")
            # Stationary block-diag weights: s_sb[p, chunk*128 + m], bf16,
            # padded to 2048 B/partition.
            s_sb = const_pool.tile([128, 1024], BF16)
            nc.scalar.dma_start(out=s_sb[:, : CHUNKS * 128], in_=s[:])
            # Bias: bias_sb[p, chunk] (512 B/partition)
            bias_sb = const_pool.tile([128, 128], F32)
            nc.scalar.dma_start(out=bias_sb[:], in_=bias[:])

            w0 = 0
            evict_flip = 0
            for nwin in STAGES:
                f0, f1 = w0 * 512, (w0 + nwin) * 512
                # R: partition p = 32j + 8i + c, free = window-local output
                # positions.  ONE contiguous descriptor per partition.
                R = r_pool.tile([128, nwin * 512], BF16)
                nc.sync.dma_start(out=R[:], in_=x[:, f0:f1], max_dma_last_dim=8192)

                for chunk in range(CHUNKS):
                    lhsT = s_sb[:, chunk * 128 : (chunk + 1) * 128]
                    stg = stage_pool.tile([128, nwin * 512], F32)
                    # Up-to-4-bank PSUM tiles: <=4 matmuls fill one, ONE wide
                    # eviction drains it (amortizes the ~0.5us/instr ACT/DVE
                    # overhead over 2048 elements).
                    for g0 in range(0, nwin, 4):
                        gw = min(4, nwin - g0)
                        ps = psum_pool.tile([128, gw * 512], F32, tag="ps")
                        for wg in range(gw):
                            w = g0 + wg
                            nc.tensor.matmul(
                                ps[:, wg * 512 : (wg + 1) * 512],
                                lhsT,
                                R[:, w * 512 : (w + 1) * 512],
                                start=True,
                                stop=True,
                            )
                        out_sl = stg[:, g0 * 512 : (g0 + gw) * 512]
                        evict_flip ^= 1
                        if evict_flip:
                            nc.scalar.activation(
                                out_sl,
                                ps[:],
                                mybir.ActivationFunctionType.Identity,
                                bias=bias_sb[:, chunk : chunk + 1],
                            )
                        else:
                            nc.vector.tensor_scalar_add(
                                out_sl, ps[:], bias_sb[:, chunk : chunk + 1]
                            )
                    # stg partition p -> y channel 128*chunk + p (affine!)
                    # Full-width 128-partition DMAs, alternating between
                    # the ACT and POOL HWDGE rings so two descriptor
                    # streams feed the SDMA engines (loads stay on SP).
                    # Split each store into column halves on BOTH rings:
                    # per-tile completion latency halves (rings drain in
                    # parallel), so stg buffers recycle faster.
                    half = (nwin * 512) // 2
                    nc.scalar.dma_start(
                        out=y_v[128 * chunk : 128 * (chunk + 1), f0 : f0 + half],
                        in_=stg[:, 0:half],
                    )
                    nc.gpsimd.dma_start(
                        out=y_v[128 * chunk : 128 * (chunk + 1), f0 + half : f1],
                        in_=stg[:, half : nwin * 512],
                    )
                w0 += nwin
    nc.compile()
    return nc


def _get_nc():
    global _NC_CACHE
    if _NC_CACHE is None:
        _NC_CACHE = _build_nc()
    return _NC_CACHE


def _host_prep(W, b):
    # Stationary chunk t computes GLOBAL output channels g = 128t + m
    # (m = psum partition).  g maps to input channel c = g//96 and conv
    # output channel oc = g%96, so psum partition <-> y channel is affine
    # and the store DMA is a full-width 128-partition transfer.
    # K index k = 32j + 8i + c:  S[t, k, m] = W[oc(g), 0, i, j]
    W2 = np.ascontiguousarray(W, dtype=np.float32).reshape(EMBED, PATCH, PATCH)
    S = np.zeros((CHUNKS, 128, 128), np.float32)
    m = np.arange(128)
    for t in range(CHUNKS):
        g = 128 * t + m
        c = g // EMBED
        oc = g % EMBED
        for i in range(PATCH):
            for j in range(PATCH):
                S[t][32 * j + 8 * i + c, m] = W2[oc, i, j]
    b = np.asarray(b, dtype=np.float32)
    # bias_pad[p, t] = b[(128t+p) % 96]  (padded to [128, 128])
    bias_pad = np.zeros((128, 128), np.float32)
    for t in range(CHUNKS):
        bias_pad[:, t] = b[(128 * t + m) % EMBED]
    # [chunk, k, m] -> [k, chunk*128+m] so the SBUF load is one contiguous
    # 1.5 KB run per partition (128 descriptors instead of 768 x 256 B)
    S = S.transpose(1, 0, 2).reshape(128, CHUNKS * 128)
    return np.ascontiguousarray(S).astype(ml_dtypes.bfloat16), bias_pad


def _prep_inputs(rgb, hs, dem, W, b):
    x16 = np.empty((N_CORES, C, H, H), ml_dtypes.bfloat16)
    x16[:, :3] = np.asarray(rgb)
    x16[:, 3:7] = np.asarray(hs)
    x16[:, 7:] = np.asarray(dem)
    # Patch-decimated layout: x'[core, 32j+8i+c, hp*128+m] =
    # x16[core, c, 4*hp+i, 4*m+j]
    xs = np.ascontiguousarray(
        x16.reshape(N_CORES, C, HP, PATCH, HP, PATCH)
        .transpose(0, 5, 3, 1, 2, 4)
        .reshape(N_CORES, 128, NPOS)
    )
    S, bias_mat = _host_prep(W, b)
    return [
        {"x": xs[core], "s": S, "bias": bias_mat} for core in range(N_CORES)
    ]


def _timing_setup(inputs):
    """Build (nc, in_maps) exactly as kernel() would — for test.py --time."""
    in_maps = _prep_inputs(
        inputs["rgb"], inputs["hs"], inputs["dem"], inputs["W"], inputs["b"]
    )
    return _get_nc(), in_maps


def kernel(rgb, hs, dem, W, b):
    in_maps = _prep_inputs(rgb, hs, dem, W, b)
    nc = _get_nc()
    res = run_bass_kernel_spmd(nc, in_maps, list(range(N_CORES)))
    return np.stack([res.results[core]["y"] for core in range(N_CORES)], axis=0)


# revision 23
# speedup vs baseline: 1.0629x; 1.0629x over previous
"""ChannelPatchEmbed kernel for Trainium2 (8 NeuronCores, batch-parallel).

Computation: concat 8 single-feature channels -> each 512x512 image goes
through the SAME 1->96 conv (4x4 patches, stride 4) + bias.
Output: [8, 768, 128, 128] f32.

Strategy per core (1 batch sample per core):
  - GEMM formulation: K = (j, i, c) = 4*4*8 = 128 on the contraction
    partitions, block-diagonal stationary S (6 chunks of 16 output
    channels x 8 input channels = 128 M) -> one K=128 matmul yields 128
    output channels per 512-position window.
  - Patch-decimated input layout (host pre-shuffle): x'[32j+8i+c,
    hp*128+m] = image[c, 4*hp+i, 4*m+j].  Every pixel appears exactly
    once, so the image loads from HBM ONCE (4.2 MB bf16) as full
    128-partition DMAs with contiguous multi-KB descriptors, and the
    matmul rhs is a contiguous 512-wide SBUF slice (no stride-4 read,
    no shifted on-chip copies).
  - Each store is issued as column pieces on multiple HWDGE rings
    concurrently (ACT+POOL for the first two stages, ACT+POOL+SP
    thirds once the sync ring has drained the loads): per-tile
    completion latency drops ~2-3x, so the 5 staging buffers recycle
    faster — relieving slow-mode jams where store completions gate
    evictions.  Won paired A/B 3-of-3 vs 2-way split (-6.9us mean).
  - bf16: inputs and W are host-cast to bf16 (tolerance 2e-2, measured
    rel err ~3.3e-3); PE runs at full rate, PSUM accumulates f32.
  - Bias is fused into wide 4-bank PSUM->SBUF evictions (ACT/DVE
    alternating); stores are full-width 128-partition DMAs alternating
    between the ACT and POOL HWDGE rings with 8-16 KB descriptors.
  - Graduated stage sizes (STAGES windows of 512 positions) shorten the
    load ramp at the start; stage bufs=5 caps the evicted-but-unstored
    backlog so the post-compute store drain stays short while still
    absorbing HBM-arbitration jitter from the other 7 cores.  One R
    buffer per stage (rin bufs=6) lets every load prefetch arbitrarily
    far ahead, cushioning runs where the load phase loses HBM
    arbitration to the other cores.
"""

import sys

import numpy as np

if "/opt/trn_rl_repo" not in sys.path:
    sys.path.insert(0, "/opt/trn_rl_repo")

import ml_dtypes

import concourse.bacc as bacc
import concourse.mybir as mybir
import concourse.tile as tile
from concourse.bass_utils import run_bass_kernel_spmd

F32 = mybir.dt.float32
BF16 = mybir.dt.bfloat16

N_CORES = 8
C = 8            # input channels per sample (3 rgb + 4 hs + 1 dem)
H = 512          # image height/width
PATCH = 4
HP = H // PATCH  # 128 patches per side
NPOS = HP * HP   # 16384 output positions per image
EMBED = 96
CHUNKS = 6       # 96*8 = 768 output channels in chunks of 128
# windows (512 output positions each) per pipeline stage; 32 windows
# total.  Small first stage -> first store flows after ~1us of load;
# tapered last stages -> short drain after the final eviction.
STAGES = [2, 4, 8, 8, 6, 4]

_NC_CACHE = None


def _build_nc():
    nc = bacc.Bacc("TRN2", target_bir_lowering=False, detect_race_conditions=False)
    x = nc.dram_tensor("x", [128, NPOS], BF16, kind="ExternalInput")
    s = nc.dram_tensor("s", [128, CHUNKS * 128], BF16, kind="ExternalInput")
    bias = nc.dram_tensor("bias", [128, 128], F32, kind="ExternalInput")
    y = nc.dram_tensor("y", [C * EMBED, HP, HP], F32, kind="ExternalOutput")

    y_v = y.rearrange("ch h w -> ch (h w)")  # [768, 16384]

    with tile.TileContext(nc) as tc:
        with (
            tc.tile_pool(name="const", bufs=1) as const_pool,
            tc.tile_pool(name="rin", bufs=6) as r_pool,
            tc.tile_pool(name="stage", bufs=5) as stage_pool,
            tc.tile_pool(name="psum", bufs=2, space="PSUM") as psum_pool,
        ):
            # Pad so every subsequent tile is 512 B-aligned: the framework's
            # const-scalar region ends at +128 B, and SDMA's sub-512B write
            # path does RMW on 512 B granules — cross-tensor granule sharing
            # between concurrent DMA writers would corrupt data.
            _align_pad = const_pool.tile([128, 96], F32, tag="align_pad")
            # Stationary block-diag weights: s_sb[p, chunk*128 + m], bf16,
            # padded to 2048 B/partition.
            s_sb = const_pool.tile([128, 1024], BF16)
            nc.scalar.dma_start(out=s_sb[:, : CHUNKS * 128], in_=s[:])
            # Bias: bias_sb[p, chunk] (512 B/partition)
            bias_sb = const_pool.tile([128, 128], F32)
            nc.scalar.dma_start(out=bias_sb[:], in_=bias[:])

            # All R loads issued up front on the sync ring (one buffer per
            # stage).  Same per-ring order as inline issue, but it lets the
            # sync ring serve as a third store lane after loads drain.
            R_all = []
            w0 = 0
            for nwin in STAGES:
                f0, f1 = w0 * 512, (w0 + nwin) * 512
                R = r_pool.tile([128, nwin * 512], BF16)
                nc.sync.dma_start(out=R[:], in_=x[:, f0:f1], max_dma_last_dim=8192)
                R_all.append(R)
                w0 += nwin

            w0 = 0
            evict_flip = 0
            for si, nwin in enumerate(STAGES):
                f0, f1 = w0 * 512, (w0 + nwin) * 512
                R = R_all[si]

                for chunk in range(CHUNKS):
                    lhsT = s_sb[:, chunk * 128 : (chunk + 1) * 128]
                    stg = stage_pool.tile([128, nwin * 512], F32)
                    # Up-to-4-bank PSUM tiles: <=4 matmuls fill one, ONE wide
                    # eviction drains it (amortizes the ~0.5us/instr ACT/DVE
                    # overhead over 2048 elements).
                    for g0 in range(0, nwin, 4):
                        gw = min(4, nwin - g0)
                        ps = psum_pool.tile([128, gw * 512], F32, tag="ps")
                        for wg in range(gw):
                            w = g0 + wg
                            nc.tensor.matmul(
                                ps[:, wg * 512 : (wg + 1) * 512],
                                lhsT,
                                R[:, w * 512 : (w + 1) * 512],
                                start=True,
                                stop=True,
                            )
                        out_sl = stg[:, g0 * 512 : (g0 + gw) * 512]
                        evict_flip ^= 1
                        if evict_flip:
                            nc.scalar.activation(
                                out_sl,
                                ps[:],
                                mybir.ActivationFunctionType.Identity,
                                bias=bias_sb[:, chunk : chunk + 1],
                            )
                        else:
                            nc.vector.tensor_scalar_add(
                                out_sl, ps[:], bias_sb[:, chunk : chunk + 1]
                            )
                    # stg partition p -> y channel 128*chunk + p (affine!)
                    # Full-width 128-partition DMAs, alternating between
                    # the ACT and POOL HWDGE rings so two descriptor
                    # streams feed the SDMA engines (loads stay on SP).
                    # Split each store into column halves on BOTH rings:
                    # per-tile completion latency halves (rings drain in
                    # parallel), so stg buffers recycle faster.
                    rows = slice(128 * chunk, 128 * (chunk + 1))
                    if si < 2:
                        half = (nwin * 512) // 2
                        nc.scalar.dma_start(
                            out=y_v[rows, f0 : f0 + half], in_=stg[:, 0:half]
                        )
                        nc.gpsimd.dma_start(
                            out=y_v[rows, f0 + half : f1],
                            in_=stg[:, half : nwin * 512],
                        )
                    else:
                        a = max(512, (nwin // 3) * 512)
                        cuts = [0, a, 2 * a, nwin * 512]
                        for (c0, c1), q in zip(
                            zip(cuts, cuts[1:]), (nc.scalar, nc.gpsimd, nc.sync)
                        ):
                            q.dma_start(
                                out=y_v[rows, f0 + c0 : f0 + c1],
                                in_=stg[:, c0:c1],
                            )
                w0 += nwin
    nc.compile()
    return nc


def _get_nc():
    global _NC_CACHE
    if _NC_CACHE is None:
        _NC_CACHE = _build_nc()
    return _NC_CACHE


def _host_prep(W, b):
    # Stationary chunk t computes GLOBAL output channels g = 128t + m
    # (m = psum partition).  g maps to input channel c = g//96 and conv
    # output channel oc = g%96, so psum partition <-> y channel is affine
    # and the store DMA is a full-width 128-partition transfer.
    # K index k = 32j + 8i + c:  S[t, k, m] = W[oc(g), 0, i, j]
    W2 = np.ascontiguousarray(W, dtype=np.float32).reshape(EMBED, PATCH, PATCH)
    S = np.zeros((CHUNKS, 128, 128), np.float32)
    m = np.arange(128)
    for t in range(CHUNKS):
        g = 128 * t + m
        c = g // EMBED
        oc = g % EMBED
        for i in range(PATCH):
            for j in range(PATCH):
                S[t][32 * j + 8 * i + c, m] = W2[oc, i, j]
    b = np.asarray(b, dtype=np.float32)
    # bias_pad[p, t] = b[(128t+p) % 96]  (padded to [128, 128])
    bias_pad = np.zeros((128, 128), np.float32)
    for t in range(CHUNKS):
        bias_pad[:, t] = b[(128 * t + m) % EMBED]
    # [chunk, k, m] -> [k, chunk*128+m] so the SBUF load is one contiguous
    # 1.5 KB run per partition (128 descriptors instead of 768 x 256 B)
    S = S.transpose(1, 0, 2).reshape(128, CHUNKS * 128)
    return np.ascontiguousarray(S).astype(ml_dtypes.bfloat16), bias_pad


def _prep_inputs(rgb, hs, dem, W, b):
    x16 = np.empty((N_CORES, C, H, H), ml_dtypes.bfloat16)
    x16[:, :3] = np.asarray(rgb)
    x16[:, 3:7] = np.asarray(hs)
    x16[:, 7:] = np.asarray(dem)
    # Patch-decimated layout: x'[core, 32j+8i+c, hp*128+m] =
    # x16[core, c, 4*hp+i, 4*m+j]
    xs = np.ascontiguousarray(
        x16.reshape(N_CORES, C, HP, PATCH, HP, PATCH)
        .transpose(0, 5, 3, 1, 2, 4)
        .reshape(N_CORES, 128, NPOS)
    )
    S, bias_mat = _host_prep(W, b)
    return [
        {"x": xs[core], "s": S, "bias": bias_mat} for core in range(N_CORES)
    ]


def _timing_setup(inputs):
    """Build (nc, in_maps) exactly as kernel() would — for test.py --time."""
    in_maps = _prep_inputs(
        inputs["rgb"], inputs["hs"], inputs["dem"], inputs["W"], inputs["b"]
    )
    return _get_nc(), in_maps


def kernel(rgb, hs, dem, W, b):
    in_maps = _prep_inputs(rgb, hs, dem, W, b)
    nc = _get_nc()
    res = run_bass_kernel_spmd(nc, in_maps, list(range(N_CORES)))
    return np.stack([res.results[core]["y"] for core in range(N_CORES)], axis=0)
